# revision 53
# baseline (speedup 1.0000x reference)
"""Trainium2 Bass kernel for an AttentionBlock (GroupNorm + 4-head self-attention + proj).

Sharding: 8 cores = 4 batches x 2 head-pairs. Core c handles batch c//2, heads
{2j, 2j+1} where j = c%2. Each core: groupnorm of x[b] (duplicated across the
pair of cores), QKV for its 128 feature channels, transposed-score flash
attention (no max subtraction -- scores are ~N(0,1)), partial projection.
Host sums the two partial projections per batch and adds residual + proj bias.

Layout on device: features/keys on partitions, tokens on free dim.
  Q, K: bf16 (128 = 2x64 head dims, 4096 tokens)
  VT:   fp8-e5m2 token-major tiles (128 tokens, [V_A(64)|1|V_B(64)|1]) --
        attention output AND softmax denominator in one pass.
  probs: fp8-e5m2, written per 128-key tile by either
        - ACT: activation(Exp, scale=1/8) with e5m2 output, or
        - DVE: Schraudolph bit-trick: u8 = rne(s*log2(e)/2 + 60) IS the e5m2
          bit pattern of exp(s/8) (linear-interp exp2; sat-at-0 kills the
          negative tail). This splits the softmax stream across two engines.
  PV:   DoubleRow fp8 matmuls: one matmul consumes TWO key tiles (256-deep
        contraction) at 0.5 cycles/row -> 4x fewer PE cycles than bf16.
"""
import sys

sys.path.insert(0, "/opt/trn_rl_repo")

import numpy as np

import concourse.bacc as bacc
import concourse.mybir as mybir
import concourse.tile as tile
from concourse import bass_utils

F32 = mybir.dt.float32
F32R = mybir.dt.float32r
BF16 = mybir.dt.bfloat16
E5 = mybir.dt.float8e5
E4 = mybir.dt.float8e4
U8 = mybir.dt.uint8
AF = mybir.ActivationFunctionType
ALU = mybir.AluOpType
AX = mybir.AxisListType
DR = mybir.MatmulPerfMode.DoubleRow

B, C, H, W = 4, 256, 64, 64
N = H * W                  # 4096 tokens
NG = 8                     # groupnorm groups
GSZ = C // NG              # 32 channels per group
NQC = 8                    # query chunks of 512
QC = 512
NKT = 32                   # key tiles of 128
KT = 128
NPR = NKT // 2             # kt pairs
INV_GN = 1.0 / GSZ         # stats are per-partition means
SCALE = 1.0 / 8.0          # hd^-0.5
A_SCH = np.log2(np.e) / 8.0 * 4.0   # schraudolph mult (folds 1/8 score scale)
B_SCH = 60.0                        # e5m2 exponent bias 15 << 2

# schedule knobs (auto-tuned via TimelineSim):
#   dve_n:    DVE (schraudolph) exp tiles per qc [qc0, rest]
#   dve_lo:   first kt eligible for DVE exps [qc0, rest]
#   fb_pos:   sub-index (2*pair+sub) where finish_b of the prev qc runs
#   fa_pos:   sub-index where the finish_a op-list starts (one op per sub)
#   pop_lo:   first pair for steady PV pops
#   pop_len:  queue length threshold for steady pops
KNOBS = dict(dve_n=(12, 14), dve_lo=(3, 5), fb_pos=15, fa_pos=5,
             pop_lo=2, pop_len=3, qk_mover="act", vt_mover="dve",
             o_copy="dve", y_copy="dve", dve_hi_last=32, q_pos=16,
             sched_mode="kt", wq="gpsimd", kshq=("sync", "gpsimd"),
             qshq=("sync", "gpsimd"), vt_act_n=4, k_mv_dve=3, vt_split=32)

_CACHE: dict = {}


def _sched_dve(qc, kt):
    n = KNOBS["dve_n"][0 if qc == 0 else 1]
    lo = KNOBS["dve_lo"][0 if qc == 0 else 1]
    if KNOBS["sched_mode"] == "pair":
        npr = max(1, n // 2)
        pr, lop = kt // 2, (lo + 1) // 2
        if n <= 0 or pr < lop:
            return False
        if qc == NQC - 1 and kt >= KNOBS["dve_hi_last"]:
            return False
        span = 16 - lop
        step = span / npr
        return int((pr - lop) // step) != int((pr - lop - 1) // step) if pr > lop else True
    if n <= 0 or kt < lo:
        return False
    if qc == NQC - 1 and kt >= KNOBS["dve_hi_last"]:
        return False
    span = 32 - lo
    step = span / n
    return int((kt - lo) // step) != int((kt - lo - 1) // step) if kt > lo else True


def _build():
    nc = bacc.Bacc("TRN2", target_bir_lowering=False, debug=False,
                   enable_asserts=False)

    xb = nc.dram_tensor("xb", [8, 128, 1024], BF16, kind="ExternalInput")
    wslb = nc.dram_tensor("wslb", [2, 128, 400], BF16, kind="ExternalInput")
    csts = nc.dram_tensor("csts", [128, 10], F32, kind="ExternalInput")
    bv16 = nc.dram_tensor("bv16", [1, 144], BF16, kind="ExternalInput")
    selt = nc.dram_tensor("selt", [4, 128], F32, kind="ExternalInput")
    wpt = nc.dram_tensor("wpt", [128, 256], BF16, kind="ExternalInput")
    yp = nc.dram_tensor("yp", [16, 128, 512], F32, kind="ExternalOutput")

    with tile.TileContext(nc) as tc:
        with (
            tc.tile_pool(name="cst", bufs=1) as cst,
            tc.tile_pool(name="big", bufs=1) as big,
            tc.tile_pool(name="pp", bufs=10) as pp,
            tc.tile_pool(name="sm", bufs=3) as sm,
            tc.tile_pool(name="yy", bufs=4) as yy,
            tc.tile_pool(name="dr", bufs=4, space="DRAM") as dr,
            tc.tile_pool(name="ps", bufs=3, space="PSUM") as ps,
            tc.tile_pool(name="po", bufs=2, space="PSUM") as po,
        ):
            # ---- constants ----
            W0 = cst.tile([128, 400], BF16, tag="w0")
            W1 = cst.tile([128, 400], BF16, tag="w1")
            WP = cst.tile([128, 256], BF16, tag="wp")
            CST = cst.tile([128, 10], F32, tag="cst")
            BQK = CST[:, 0:2]
            GAM = CST[:, 2:4]
            BET = CST[:, 4:6]
            SEL = CST[:, 6:10]
            BV = cst.tile([1, 144], BF16, tag="bv")
            SELT = cst.tile([4, 128], F32, tag="selt")
            ONE = cst.tile([1, 128], F32, tag="one")
            ONEB = cst.tile([1, 128], BF16, tag="oneb")
            EPS = cst.tile([128, 1], F32, tag="eps")

            # prime the sqrt act table before the stats activations (all of
            # Identity/Square live in every table) so the groupnorm Sqrt and
            # the stats run without any mid-prologue table reloads.
            WARM = cst.tile([1, 1], F32, tag="warm")
            nc.vector.memset(WARM[:], 1.0)
            nc.scalar.activation(WARM[:], WARM[:], AF.Sqrt)

            # ---- load x (chunked, stats via one-pass bn_stats) ----
            NCH = 8
            CH = N // NCH   # 512
            X = [big.tile([128, N], BF16, tag=f"x{cc}", name=f"X{cc}") for cc in range(2)]
            Hb = [big.tile([128, N], BF16, tag=f"hb{cc}", name=f"Hb{cc}") for cc in range(2)]
            BNS = [cst.tile([128, NCH * 6], F32, tag=f"bns{cc}", name=f"BNS{cc}") for cc in range(2)]
            MV = [cst.tile([128, 2], F32, tag=f"mv{cc}", name=f"MV{cc}") for cc in range(2)]
            ST = [cst.tile([128, 2], F32, tag=f"st{cc}", name=f"ST{cc}") for cc in range(2)]
            GS = cst.tile([4, 4], F32, tag="gs")
            gs_ps = po.tile([4, 4], F32, tag="po")
            SX = cst.tile([128, 4], F32, tag="sx")
            SQ = cst.tile([128, 4], F32, tag="sq")
            for i in range(4):
                for cc in range(2):
                    dsl = slice(i * 1024, (i + 1) * 1024)
                    xq = nc.sync if cc == 0 else nc.gpsimd
                    xq.dma_start(X[cc][:, dsl], xb.ap()[cc * 4 + i])
                    for h in range(2):
                        j = 2 * i + h
                        sl = slice(j * CH, (j + 1) * CH)
                        if cc == 1 and j < 4:
                            # half-1 stats on ACT (idle early); scratch into
                            # Hb[1] (overwritten later by real Hb)
                            nc.scalar.activation(
                                Hb[1][:, sl], X[1][:, sl], AF.Identity,
                                accum_out=SX[:, j:j + 1])
                            nc.scalar.activation(
                                Hb[1][:, sl], X[1][:, sl], AF.Square,
                                accum_out=SQ[:, j:j + 1])
                        else:
                            nc.vector.bn_stats(BNS[cc][:, 6 * j:6 * j + 6],
                                               X[cc][:, sl])
            # weights & consts (needed later than x)
            nc.vector.memset(EPS[:], 1e-5)
            nc.vector.memset(ONE[:], 1.0)
            nc.vector.memset(ONEB[:], 1.0)
            WQ = {"sync": nc.sync, "gpsimd": nc.gpsimd, "scalar": nc.scalar}[KNOBS["wq"]]
            WQ.dma_start(CST[:], csts.ap())
            WQ.dma_start(BV[:], bv16.ap())
            WQ.dma_start(SELT[:], selt.ap())
            WQ.dma_start(W0[:], wslb.ap()[0])
            WQ.dma_start(W1[:], wslb.ap()[1])
            WQ.dma_start(WP[:], wpt.ap())
            for cc in range(2):
                if cc == 0:
                    nc.vector.bn_aggr(MV[0][:], BNS[0][:])
                    # ST = [mean_p, E[x^2]_p]
                    nc.vector.tensor_mul(ST[0][:, 1:2], MV[0][:, 0:1], MV[0][:, 0:1])
                    nc.vector.tensor_add(ST[0][:, 1:2], ST[0][:, 1:2], MV[0][:, 1:2])
                    nc.vector.tensor_copy(ST[0][:, 0:1], MV[0][:, 0:1])
                else:
                    # merge ACT sums (slices 0-3) with bn stats (slices 4-7)
                    nc.vector.bn_aggr(MV[1][:], BNS[1][:, 24:48])
                    sxs = cst.tile([128, 1], F32, tag="sxs")
                    sqs = cst.tile([128, 1], F32, tag="sqs")
                    nc.vector.reduce_sum(sxs[:], SX[:], axis=AX.X)
                    nc.vector.reduce_sum(sqs[:], SQ[:], axis=AX.X)
                    # mean_p = 0.5*mean_bn + sxs/4096
                    nc.vector.tensor_scalar_mul(ST[1][:, 0:1], MV[1][:, 0:1], 0.5)
                    nc.vector.tensor_scalar_mul(sxs[:], sxs[:], 1.0 / 4096.0)
                    nc.vector.tensor_add(ST[1][:, 0:1], ST[1][:, 0:1], sxs[:])
                    # E2_p = 0.5*(var_bn + mean_bn^2) + sqs/4096
                    nc.vector.tensor_mul(ST[1][:, 1:2], MV[1][:, 0:1], MV[1][:, 0:1])
                    nc.vector.tensor_add(ST[1][:, 1:2], ST[1][:, 1:2], MV[1][:, 1:2])
                    nc.vector.tensor_scalar_mul(ST[1][:, 1:2], ST[1][:, 1:2], 0.5)
                    nc.vector.tensor_scalar_mul(sqs[:], sqs[:], 1.0 / 4096.0)
                    nc.vector.tensor_add(ST[1][:, 1:2], ST[1][:, 1:2], sqs[:])
                nc.tensor.matmul(gs_ps[:, 2 * cc:2 * cc + 2], SEL,
                                 ST[cc][:], start=True, stop=True)
                nc.vector.tensor_copy(GS[:, 2 * cc:2 * cc + 2],
                                      gs_ps[:, 2 * cc:2 * cc + 2])

            # per-channel scale/shift: s = gamma/sqrt(var+eps), t = beta - mean*s
            gn_st = []
            for cc in range(2):
                pc_ps = po.tile([128, 2], F32, tag="po")
                nc.tensor.matmul(pc_ps[:], SELT[:], GS[:, 2 * cc:2 * cc + 2],
                                 start=True, stop=True)
                mean = cst.tile([128, 1], F32, tag=f"mean{cc}")
                var = cst.tile([128, 1], F32, tag=f"var{cc}")
                sd = cst.tile([128, 1], F32, tag=f"sd{cc}")
                s_t = cst.tile([128, 1], F32, tag=f"s{cc}")
                t_t = cst.tile([128, 1], F32, tag=f"t{cc}")
                nc.vector.tensor_scalar_mul(mean[:], pc_ps[:, 0:1], INV_GN)
                nc.vector.tensor_scalar_mul(var[:], pc_ps[:, 1:2], INV_GN)
                # var = E[x^2] - mean^2
                nc.vector.scalar_tensor_tensor(
                    out=sd[:], in0=mean[:], scalar=-1.0, in1=mean[:],
                    op0=ALU.mult, op1=ALU.mult)
                nc.vector.tensor_add(var[:], var[:], sd[:])
                nc.scalar.activation(sd[:], var[:], AF.Sqrt, bias=EPS[:])
                nc.vector.reciprocal(s_t[:], sd[:])
                nc.vector.tensor_mul(s_t[:], s_t[:], GAM[:, cc:cc + 1])
                nc.vector.scalar_tensor_tensor(
                    out=t_t[:], in0=mean[:], scalar=-1.0, in1=s_t[:],
                    op0=ALU.mult, op1=ALU.mult)
                nc.vector.tensor_add(t_t[:], t_t[:], BET[:, cc:cc + 1])
                gn_st.append((s_t, t_t))

            # h = x*s + t (bf16): all on DVE -- all-SBUF 2-byte TensorScalar
            # hits the 4x DVE mode (~327ns/slice)
            nc.scalar.activation(WARM[:], WARM[:], AF.Exp)  # preload exp table
            def emit_hb(i):
                sl = slice(i * 1024, (i + 1) * 1024)
                for cc in range(2):
                    s_t, t_t = gn_st[cc]
                    nc.vector.tensor_scalar(
                        out=Hb[cc][:, sl], in0=X[cc][:, sl], scalar1=s_t[:],
                        scalar2=t_t[:], op0=ALU.mult, op1=ALU.add)
            emit_hb(0)

            # ---- QKV ----
            # Q2/K2: fp8-e4m3, head-dim split across 2 DoubleRow k-subtiles:
            # partition r (0..31), free = h*8192 + t*4096 + token, where the
            # original feature index is h*64 + t*32 + r. QK then runs as one
            # dual-pumped fp8 matmul per (kt, head): 107ns instead of 427.
            Q2 = big.tile([32, 4 * N], E4, tag="q2")
            K2 = big.tile([32, 4 * N], E4, tag="k2")
            VT = big.tile([128, NKT * 144], E5, tag="vt")

            Q8 = [big.tile([128, QC], E4, tag=f"q8_{i}", name=f"Q8_{i}")
                  for i in range(2)]
            K8 = [big.tile([128, QC], E4, tag=f"k8_{i}", name=f"K8_{i}")
                  for i in range(2)]

            def _shuffle(dst2, stage, ch, q0, q1):
                # (128, 512) staging -> (32, [h][t] chunk): 4 partition-block
                # copies spread over two DMA queues
                for h in range(2):
                    for t in range(2):
                        off = (h * 2 + t) * N + ch * QC
                        q = q0 if (h * 2 + t) % 2 == 0 else q1
                        q.dma_start(dst2[:, off:off + QC],
                                    stage[h * 64 + t * 32:h * 64 + t * 32 + 32, :])

            def emit_q_chunk(ch):
                tok = slice(ch * QC, (ch + 1) * QC)
                q_ps = ps.tile([128, QC], F32, tag="s", name=f"q_ps{ch}")
                nc.tensor.matmul(q_ps[:], W0[:, 0:128], Hb[0][:, tok],
                                 start=True, stop=False)
                nc.tensor.matmul(q_ps[:], W1[:, 0:128], Hb[1][:, tok],
                                 start=False, stop=True)
                stage = Q8[ch % 2]
                if KNOBS["qk_mover"] == "dve":
                    nc.vector.tensor_scalar(out=stage[:], in0=q_ps[:],
                                            scalar1=BQK[:, 0:1], scalar2=None,
                                            op0=ALU.add)
                else:
                    nc.scalar.activation(stage[:], q_ps[:], AF.Identity,
                                         bias=BQK[:, 0:1])
                qs = {"sync": nc.sync, "gpsimd": nc.gpsimd}
                _shuffle(Q2, stage, ch, qs[KNOBS["qshq"][0]], qs[KNOBS["qshq"][1]])

            def emit_k_chunk(ch):
                tok = slice(ch * QC, (ch + 1) * QC)
                k_ps = ps.tile([128, QC], F32, tag="s", name=f"k_ps{ch}")
                nc.tensor.matmul(k_ps[:], W0[:, 128:256], Hb[0][:, tok],
                                 start=True, stop=False)
                nc.tensor.matmul(k_ps[:], W1[:, 128:256], Hb[1][:, tok],
                                 start=False, stop=True)
                stage = K8[ch % 2]
                if ch != 0 and ch <= KNOBS["k_mv_dve"]:
                    nc.vector.tensor_scalar(out=stage[:], in0=k_ps[:],
                                            scalar1=BQK[:, 1:2], scalar2=None,
                                            op0=ALU.add)
                elif KNOBS["qk_mover"] == "dve":
                    nc.vector.tensor_scalar(out=stage[:], in0=k_ps[:],
                                            scalar1=BQK[:, 1:2], scalar2=None,
                                            op0=ALU.add)
                else:
                    nc.scalar.activation(stage[:], k_ps[:], AF.Identity,
                                         bias=BQK[:, 1:2])
                qs = {"sync": nc.sync, "gpsimd": nc.gpsimd}
                _shuffle(K2, stage, ch, qs[KNOBS["kshq"][0]], qs[KNOBS["kshq"][1]])

            def emit_vt_tile(kt):
                tok = slice(kt * KT, (kt + 1) * KT)
                vt_ps = ps.tile([128, 144], F32, tag="s", name=f"vt_ps{kt}")
                nc.tensor.matmul(vt_ps[:], Hb[0][:, tok], W0[:, 256:400],
                                 start=True, stop=False)
                nc.tensor.matmul(vt_ps[:], Hb[1][:, tok], W1[:, 256:400],
                                 start=False, stop=False)
                # V bias (+ the denominator 1s column) via rank-1 accumulate
                nc.tensor.matmul(vt_ps[:], ONEB[0:1, :], BV[:],
                                 start=False, stop=True)
                vm = KNOBS["vt_mover"]
                n_act = KNOBS["vt_act_n"]
                if n_act and kt % max(1, NKT // n_act) == 1:
                    vm = "act"
                if vm == "dve":
                    nc.vector.tensor_copy(VT[:, kt * 144:(kt + 1) * 144], vt_ps[:])
                else:
                    nc.scalar.activation(VT[:, kt * 144:(kt + 1) * 144], vt_ps[:],
                                         AF.Copy)

            # chunk-0 QKV only needs Hb cols 0:512 -- start it right after
            # the first Hb slice, then fill in the rest of Hb
            emit_q_chunk(0)
            emit_k_chunk(0)
            for i in range(1, 4):
                emit_hb(i)

            # ---- attention + projection ----
            # qc-boundary flow (all during qc+1): PV pops lag ~3 pairs; the
            # last PVs of qc pop at qc+1 pair 0; finish_a (recips + bounce
            # DMAs) and the O->SBUF copies run at pair 1, freeing the O psum
            # banks immediately (no WAR on the bounce latency); finish_b
            # (normalize on Pool + proj + y) runs at pair 5 when the bounce
            # broadcast has landed. Neither exp stream ever blocks.
            pending_a = None
            pending_b = None
            pv_queue = []
            for qc in range(NQC):
                qs = slice(qc * QC, (qc + 1) * QC)
                O_A = po.tile([72, QC], F32, tag="po", name=f"O_A{qc}")
                O_B = po.tile([72, QC], F32, tag="po", name=f"O_B{qc}")
                for pr in range(NPR):
                    P8 = pp.tile([128, 2048], E5, tag="p", name=f"p{qc}_{pr}")
                    for sub in range(2):
                        kt = 2 * pr + sub
                        if qc == 0:
                            if kt % 4 == 0 and kt // 4 < 7:
                                emit_k_chunk(kt // 4 + 1)
                            if kt < KNOBS["vt_split"]:
                                emit_vt_tile(kt)
                        if qc == 1 and kt < NKT - KNOBS["vt_split"]:
                            # late VT tiles aren't consumed until qc1's early
                            # PV pops -- produce them here to unload qc0
                            emit_vt_tile(kt + KNOBS["vt_split"])
                        if pr <= 1 and pv_queue:
                            pv_queue.pop(0)()
                        si = 2 * pr + sub
                        if pending_a and KNOBS["fa_pos"] <= si:
                            i = si - KNOBS["fa_pos"]
                            if i < len(pending_a):
                                pending_a[i]()
                                if i == len(pending_a) - 1:
                                    pending_a = None
                        if si == KNOBS["fb_pos"] and pending_b is not None:
                            pending_b()
                            pending_b = None
                        if si == KNOBS["q_pos"] and qc < NQC - 1:
                            emit_q_chunk(qc + 1)
                        if pr >= KNOBS["pop_lo"] and len(pv_queue) >= KNOBS["pop_len"]:
                            pv_queue.pop(0)()
                        s_ps = ps.tile([128, 1024], F32, tag="s",
                                       name=f"s{qc}_{kt}")
                        if qc == 0 and kt < 8:
                            # warmup: the un-shuffled e4m3 staging tiles still
                            # hold these key chunks -- plain fp8 matmuls, no
                            # wait on the shuffle DMAs
                            ks8 = K8[(kt // 4) % 2]
                            kk = slice(kt % 4 * KT, (kt % 4 + 1) * KT)
                            nc.tensor.matmul(s_ps[:, 0:512], ks8[0:64, kk],
                                             Q8[0][0:64, :], start=True,
                                             stop=True)
                            nc.tensor.matmul(s_ps[:, 512:1024], ks8[64:128, kk],
                                             Q8[0][64:128, :], start=True,
                                             stop=True)
                        else:
                            k2v = K2[:].rearrange("p (h t c) -> h p t c", h=2, t=2)[
                                :, :, :, kt * KT:(kt + 1) * KT]
                            q2v = Q2[:].rearrange("p (h t c) -> h p t c", h=2, t=2)[
                                :, :, :, qc * QC:(qc + 1) * QC]
                            nc.tensor.matmul(s_ps[:, 0:512], k2v[0], q2v[0],
                                             start=True, stop=True, perf_mode=DR)
                            nc.tensor.matmul(s_ps[:, 512:1024], k2v[1], q2v[1],
                                             start=True, stop=True, perf_mode=DR)
                        dst = P8[:, sub * 1024:(sub + 1) * 1024]
                        if _sched_dve(qc, kt):
                            nc.vector.tensor_scalar(
                                out=dst.bitcast(U8), in0=s_ps[:],
                                scalar1=A_SCH, scalar2=B_SCH,
                                op0=ALU.mult, op1=ALU.add)
                        else:
                            nc.scalar.activation(dst, s_ps[:], AF.Exp,
                                                 scale=SCALE)

                    def _pv(pr=pr, P8=P8, O_A=O_A, O_B=O_B):
                        vt_ap = VT[:].rearrange("p (t x) -> p t x", t=NKT)[
                            :, 2 * pr:2 * pr + 2, :]
                        p_ap = P8[:].rearrange("p (t x) -> p t x", t=2)
                        nc.tensor.matmul(O_A[:], vt_ap[:, :, 0:72],
                                         p_ap[:, :, 0:512],
                                         start=(pr == 0), stop=(pr == NPR - 1),
                                         perf_mode=DR)
                        nc.tensor.matmul(O_B[:], vt_ap[:, :, 72:144],
                                         p_ap[:, :, 512:1024],
                                         start=(pr == 0), stop=(pr == NPR - 1),
                                         perf_mode=DR)
                    pv_queue.append(_pv)

                bcsA = sm.tile([64, QC], F32, tag="bcsa", name=f"bcsA{qc}")
                bcsB = sm.tile([64, QC], F32, tag="bcsb", name=f"bcsB{qc}")
                rA = sm.tile([1, QC], F32, tag="ra", name=f"rA{qc}")
                rB = sm.tile([1, QC], F32, tag="rb", name=f"rB{qc}")
                OsbA = sm.tile([72, QC], BF16, tag="osba", name=f"OsbA{qc}")
                OsbB = sm.tile([72, QC], BF16, tag="osbb", name=f"OsbB{qc}")

                def fa0(qc=qc, O_A=O_A, rA=rA, OsbA=OsbA):
                    nc.vector.reciprocal(rA[:], O_A[64:65, :])
                    if KNOBS["o_copy"] in ("act", "split"):
                        nc.scalar.activation(OsbA[:], O_A[:], AF.Copy)
                    else:
                        nc.vector.tensor_copy(OsbA[:], O_A[:])

                def fa1(qc=qc, O_B=O_B, rB=rB, OsbB=OsbB):
                    nc.vector.reciprocal(rB[:], O_B[64:65, :])
                    if KNOBS["o_copy"] == "dve":
                        nc.vector.tensor_copy(OsbB[:], O_B[:])
                    else:
                        nc.scalar.activation(OsbB[:], O_B[:], AF.Copy)

                def fa2(qc=qc, bcsA=bcsA, bcsB=bcsB, rA=rA, rB=rB):
                    rAd = dr.tile([1, QC], F32, tag="rad", name=f"rAd{qc}")
                    rBd = dr.tile([1, QC], F32, tag="rbd", name=f"rBd{qc}")
                    nc.sync.dma_start(rAd[:], rA[:])
                    nc.sync.dma_start(rBd[:], rB[:])
                    nc.sync.dma_start(bcsA[:], rAd[:].broadcast_to((64, QC)))
                    nc.sync.dma_start(bcsB[:], rBd[:].broadcast_to((64, QC)))

                def finish_b(qc=qc, bcsA=bcsA, bcsB=bcsB, OsbA=OsbA, OsbB=OsbB):
                    attn = sm.tile([128, QC], BF16, tag="attn", name=f"attn{qc}")
                    nc.gpsimd.tensor_mul(attn[0:64, :], OsbA[0:64, :], bcsA[:])
                    nc.gpsimd.tensor_mul(attn[64:128, :], OsbB[0:64, :], bcsB[:])
                    for half in range(2):
                        y_ps = ps.tile([128, QC], F32, tag="s", name=f"y_ps{qc}_{half}")
                        nc.tensor.matmul(y_ps[:], WP[:, half * 128:(half + 1) * 128],
                                         attn[:], start=True, stop=True)
                        y_sb = yy.tile([128, QC], F32, tag="y", name=f"y_sb{qc}_{half}")
                        yc = KNOBS["y_copy"]
                        if yc == "dve" or (yc == "split" and half == 1):
                            nc.vector.tensor_copy(y_sb[:], y_ps[:])
                        else:
                            nc.scalar.activation(y_sb[:], y_ps[:], AF.Copy)
                        nc.sync.dma_start(yp.ap()[half * 8 + qc], y_sb[:])

                if qc < NQC - 1:
                    pending_a = [fa0, fa1, fa2]
                    pending_b = finish_b
            while pv_queue:
                pv_queue.pop(0)()
            # tail (last qc): shortest-latency serial chain -- PE K=1
            # broadcast instead of the DMA bounce, normalize straight from
            # the O psum (no evacuation), copies split across ACT/DVE.
            rA_t = sm.tile([1, QC], F32, tag="ra", name="rA_t")
            rB_t = sm.tile([1, QC], F32, tag="rb", name="rB_t")
            nc.vector.reciprocal(rA_t[:], O_A[64:65, :])
            nc.vector.reciprocal(rB_t[:], O_B[64:65, :])
            bc_ps = ps.tile([128, 1024], F32, tag="s", name="bc_tail")
            nc.tensor.matmul(bc_ps[0:64, 0:512], ONE[0:1, 0:64],
                             rA_t[:], start=True, stop=True)
            nc.tensor.matmul(bc_ps[64:128, 0:512], ONE[0:1, 0:64],
                             rB_t[:], start=True, stop=True)
            bcsA_t = sm.tile([64, QC], F32, tag="bcsa", name="bcsA_t")
            bcsB_t = sm.tile([64, QC], F32, tag="bcsb", name="bcsB_t")
            nc.vector.tensor_copy(bcsA_t[:], bc_ps[0:64, 0:512])
            nc.scalar.activation(bcsB_t[:], bc_ps[64:128, 0:512], AF.Copy)
            attn_t = sm.tile([128, QC], BF16, tag="attn", name="attn_t")
            nc.vector.tensor_mul(attn_t[0:64, :], O_A[0:64, :], bcsA_t[:])
            nc.vector.tensor_mul(attn_t[64:128, :], O_B[0:64, :], bcsB_t[:])
            for half in range(2):
                y_ps = ps.tile([128, QC], F32, tag="s", name=f"y_ps_t{half}")
                nc.tensor.matmul(y_ps[:], WP[:, half * 128:(half + 1) * 128],
                                 attn_t[:], start=True, stop=True)
                y_sb = yy.tile([128, QC], F32, tag="y", name=f"y_sb_t{half}")
                if half == 0:
                    nc.scalar.activation(y_sb[:], y_ps[:], AF.Copy)
                    nc.sync.dma_start(yp.ap()[half * 8 + NQC - 1], y_sb[:])
                else:
                    nc.vector.tensor_copy(y_sb[:], y_ps[:])
                    nc.gpsimd.dma_start(yp.ap()[half * 8 + NQC - 1], y_sb[:])

    nc.compile()
    return nc


def _get_nc():
    if "nc" not in _CACHE:
        _CACHE["nc"] = _build()
    return _CACHE["nc"]


def build_in_maps(x, gn_gamma, gn_beta, w_qkv, b_qkv, w_proj):
    import ml_dtypes
    sel_np = np.zeros((128, 4), np.float32)
    for c in range(128):
        sel_np[c, c // 32] = 1.0
    selt_np = sel_np.T.copy()
    gmt_np = np.stack([gn_gamma[0:128], gn_gamma[128:256]], axis=1)
    btt_np = np.stack([gn_beta[0:128], gn_beta[128:256]], axis=1)

    in_maps = []
    for core in range(8):
        b, j = core // 2, core % 2
        r0 = 128 * j
        wsl_np = np.zeros((2, 128, 400), np.float32)
        for cc in range(2):
            cols = slice(cc * 128, (cc + 1) * 128)
            wsl_np[cc, :, 0:128] = w_qkv[r0:r0 + 128, cols].T
            wsl_np[cc, :, 128:256] = w_qkv[256 + r0:256 + r0 + 128, cols].T
            wsl_np[cc, :, 256:320] = w_qkv[512 + r0:512 + r0 + 64, cols].T
            wsl_np[cc, :, 328:392] = w_qkv[512 + r0 + 64:512 + r0 + 128, cols].T
        bqk_np = np.stack([b_qkv[r0:r0 + 128], b_qkv[256 + r0:256 + r0 + 128]],
                          axis=1)
        bv_np = np.zeros((1, 144), np.float32)
        bv_np[0, 0:64] = b_qkv[512 + r0:512 + r0 + 64]
        bv_np[0, 64] = 1.0
        bv_np[0, 72:136] = b_qkv[512 + r0 + 64:512 + r0 + 128]
        bv_np[0, 136] = 1.0
        csts_np = np.concatenate([bqk_np, gmt_np, btt_np, sel_np], axis=1)
        xq = np.ascontiguousarray(
            x[b].reshape(2, 128, 4, 1024).transpose(0, 2, 1, 3)
            .reshape(8, 128, 1024).astype(ml_dtypes.bfloat16))
        in_maps.append({
            "xb": xq,
            "wslb": np.ascontiguousarray(wsl_np.astype(ml_dtypes.bfloat16)),
            "csts": np.ascontiguousarray(csts_np),
            "bv16": np.ascontiguousarray(bv_np.astype(ml_dtypes.bfloat16)),
            "selt": selt_np,
            "wpt": np.ascontiguousarray(
                w_proj[:, r0:r0 + 128].T.astype(ml_dtypes.bfloat16)),
        })

    return in_maps


def kernel(x, gn_gamma, gn_beta, w_qkv, b_qkv, w_proj, b_proj, **_unused):
    x = np.ascontiguousarray(np.asarray(x, dtype=np.float32))
    gn_gamma = np.asarray(gn_gamma, dtype=np.float32)
    gn_beta = np.asarray(gn_beta, dtype=np.float32)
    w_qkv = np.asarray(w_qkv, dtype=np.float32)
    b_qkv = np.asarray(b_qkv, dtype=np.float32)
    w_proj = np.asarray(w_proj, dtype=np.float32)
    b_proj = np.asarray(b_proj, dtype=np.float32)

    nc = _get_nc()
    in_maps = build_in_maps(x, gn_gamma, gn_beta, w_qkv, b_qkv, w_proj)
    res = bass_utils.run_bass_kernel_spmd(nc, in_maps, core_ids=list(range(8)))
    _CACHE["last_result"] = res

    out = np.empty((B, C, N), np.float32)
    for b in range(B):
        ypsum = res.results[2 * b]["yp"] + res.results[2 * b + 1]["yp"]
        ypsum = ypsum.reshape(2, 8, 128, 512).transpose(0, 2, 1, 3).reshape(C, N)
        out[b] = ypsum + x[b].reshape(C, N) + b_proj[:, None]
    return out.reshape(B, C, H, W)
